# revision 5
# baseline (speedup 1.0000x reference)
"""Trainium2 Bass kernel for nn_AttentionBlock (B=2, T=2048, D=1024, H=16,
Dh=64, Ff=4096), SPMD across 8 NeuronCores in one NEFF launch.

Sharding:
  - Phase 1+2 (QKV projection + attention): 2 heads per core. The alibi
    tensor (256 MiB total) is read bf16, 2 heads per core.
  - AllToAll (1 MiB/core, bf16) re-shards attention output from heads to
    tokens.
  - Phase 3 (out-proj + residual + LayerNorm + MLP): 512 tokens per core.

Numerics:
  - Matmuls on the q/k path use float32r (TF32-like, ~1.5e-4) fed straight
    from fp32 HBM data; bf16 elsewhere (weights, alibi).
  - Attention computes transposed scores S^T(k,q): alibi is injected into
    PSUM via an identity matmul and Q.K^T accumulates on top, so the
    softmax input needs no extra vector work; exp runs on ScalarE
    (PSUM -> SBUF f32r); the softmax denominator falls out of a ones
    column appended to V in the attn@v matmul; 1/denom is broadcast
    across partitions via a DRAM-bounce DMA.
  - Host-side algebraic folds: 1/sqrt(Dh) into w_q, ln2_w into w_mlp_in,
    b_mlp_in via gelu's per-partition bias, b_mlp_out into a second copy
    of the residual.

kernel(**inputs) takes FULL unsharded inputs, returns the FULL output.
"""

import sys

for _p in ("/opt/trn_rl_repo", "/root/.axon_site/_ro/trn_rl_repo"):
    if _p not in sys.path:
        sys.path.insert(0, _p)

import numpy as np
import ml_dtypes

import concourse.bass as bass
import concourse.tile as tile
from concourse import bacc, mybir
from concourse.bass_utils import run_bass_kernel_spmd
from concourse.masks import make_identity

BF16 = ml_dtypes.bfloat16

B, T, D, H, Dh, FF = 2, 2048, 1024, 16, 64, 4096
NTOK = B * T            # 4096
NCORES = 8
CHUNK = NTOK // NCORES  # 512 tokens per core
HPC = H // NCORES       # 2 heads per core

F32 = mybir.dt.float32
F32R = mybir.dt.float32r
BF = mybir.dt.bfloat16
AF = mybir.ActivationFunctionType

_COMPILED = None


def _build():
    nc = bacc.Bacc("TRN2", target_bir_lowering=False, debug=False,
                   num_devices=NCORES)

    # ---- kernel I/O (per core) ----
    xT_io = nc.dram_tensor("xT", [D, NTOK], F32R, kind="ExternalInput").ap()
    wqkvT_io = nc.dram_tensor("wqkvT", [D, 384], F32R, kind="ExternalInput").ap()
    alibiT_io = nc.dram_tensor("alibiT", [HPC, T, T], BF, kind="ExternalInput").ap()
    w_outT_io = nc.dram_tensor("w_outT", [D, D], BF, kind="ExternalInput").ap()
    x_res_io = nc.dram_tensor("x_res", [CHUNK, D], F32, kind="ExternalInput").ap()
    x_res_b_io = nc.dram_tensor("x_res_b", [CHUNK, D], F32, kind="ExternalInput").ap()
    # packed as [p, ff, kk, fin] = w_mlp_in_eff[ff*128+fin, kk*128+p]
    w_inP_io = nc.dram_tensor("w_inP", [128, 32, 8, 128], BF, kind="ExternalInput").ap()
    b_inT_io = nc.dram_tensor("b_inT", [128, 32], F32, kind="ExternalInput").ap()
    w_mlp_outT_io = nc.dram_tensor("w_mlp_outT", [FF, D], BF, kind="ExternalInput").ap()
    out_io = nc.dram_tensor("out", [CHUNK, D], F32, kind="ExternalOutput").ap()

    # ---- internal DRAM ----
    cc_send = nc.dram_tensor("cc_send", [D, CHUNK], BF)
    cc_recv = nc.dram_tensor("cc_recv", [D, CHUNK], BF)
    denom_dram = nc.dram_tensor("denom_dram", [16, 512], F32)

    KT = T // 128   # 16 k-tiles per batch

    with tile.TileContext(nc) as tc:
        with tc.tile_pool(name="consts", bufs=1) as consts:
            identb = consts.tile([128, 128], BF, tag="identb")
            make_identity(nc, identb[:])
            identf = consts.tile([128, 128], F32, tag="identf")
            make_identity(nc, identf[:])
            identr = consts.tile([128, 128], F32R, tag="identr")
            nc.vector.tensor_copy(identr[:], identf[:])

            with tc.tile_pool(name="qkv", bufs=1) as qkv:
                qT = qkv.tile([128, NTOK], F32R, tag="qT")
                kT = qkv.tile([128, NTOK], F32R, tag="kT")
                # token-major v per (b, head): 16 tok-tiles of (128, 65);
                # index [:, (b*2+hl)*16 + ti, :]; col 64 = ones (denominator)
                v_all = qkv.tile([128, 64, 65], F32R, tag="v_all")
                onesf = qkv.tile([128, 64, 1], F32, tag="onesf")
                nc.vector.memset(onesf[:], 1.0)
                nc.vector.tensor_copy(v_all[:, :, 64:65], onesf[:])
                yn = [qkv.tile([64, NTOK], BF, tag=f"yn{hl}",
                               name=f"yn{hl}") for hl in range(2)]

                # ---------------- phase 1: QKV projection ----------------
                with nc.named_scope("qkvproj"), \
                     tc.tile_pool(name="p1x", bufs=1) as p1x, \
                     tc.tile_pool(name="p1w", bufs=1) as p1w, \
                     tc.tile_pool(name="p1ps", bufs=3, space="PSUM") as p1ps, \
                     tc.tile_pool(name="p1t", bufs=3) as p1t, \
                     tc.tile_pool(name="p1pt", bufs=2, space="PSUM") as p1pt:
                    wq = []
                    for kk in range(8):
                        w = p1w.tile([128, 384], F32R, tag=f"wq{kk}")
                        nc.sync.dma_start(w[:], wqkvT_io[kk * 128:(kk + 1) * 128, :])
                        wq.append(w)
                    for half in range(2):
                        xts = []
                        for kk in range(8):
                            xt = p1x.tile([128, 2048], F32R, tag=f"xt{kk}")
                            nc.sync.dma_start(
                                xt[:], xT_io[kk * 128:(kk + 1) * 128,
                                             half * 2048:(half + 1) * 2048])
                            xts.append(xt)
                        for t in range(4):
                            g = half * 4 + t
                            for m in range(3):   # q, k, v
                                ps = p1ps.tile([128, 512], F32, tag="proj")
                                for kk in range(8):
                                    nc.tensor.matmul(
                                        ps[:],
                                        wq[kk][:, m * 128:(m + 1) * 128],
                                        xts[kk][:, t * 512:(t + 1) * 512],
                                        start=(kk == 0), stop=(kk == 7))
                                if m == 0:
                                    nc.vector.tensor_copy(
                                        qT[:, g * 512:(g + 1) * 512], ps[:])
                                elif m == 1:
                                    nc.vector.tensor_copy(
                                        kT[:, g * 512:(g + 1) * 512], ps[:])
                                else:
                                    vt = p1t.tile([128, 512], F32R, tag="vt")
                                    nc.vector.tensor_copy(vt[:], ps[:])
                                    b = g // 4
                                    for j in range(4):
                                        ti = (g % 4) * 4 + j
                                        pt = p1pt.tile([128, 128], F32R,
                                                       tag="pt")
                                        nc.tensor.transpose(
                                            pt[:],
                                            vt[:, j * 128:(j + 1) * 128],
                                            identr[:])
                                        for hl in range(2):
                                            nc.vector.tensor_copy(
                                                v_all[:, (b * 2 + hl) * 16 + ti, 0:64],
                                                pt[:, hl * 64:(hl + 1) * 64])

                # ---------------- phase 2: attention ----------------
                with nc.named_scope("attn"), \
                     tc.tile_pool(name="alb", bufs=4) as albp, \
                     tc.tile_pool(name="exps", bufs=5) as expp, \
                     tc.tile_pool(name="sps", bufs=4, space="PSUM") as spsp, \
                     tc.tile_pool(name="yups", bufs=4, space="PSUM") as yupp, \
                     tc.tile_pool(name="nrm", bufs=3) as nrmp:
                    for qc in range(4):
                        yus = [yupp.tile([65, 512], F32, tag="yu",
                                         name=f"yu{qc}_{i}")
                               for i in range(4)]   # index b*2+hl
                        for kt in range(KT):
                            for hl in range(2):
                                al = albp.tile([128, 512], BF, tag="al")
                                nc.sync.dma_start(
                                    al[:],
                                    alibiT_io[hl, kt * 128:(kt + 1) * 128,
                                              qc * 512:(qc + 1) * 512])
                                for b in range(2):
                                    sp = spsp.tile([128, 512], F32, tag="sp")
                                    nc.tensor.matmul(sp[:], identb[:], al[:],
                                                     start=True, stop=False)
                                    nc.tensor.matmul(
                                        sp[:],
                                        kT[hl * 64:(hl + 1) * 64,
                                           b * T + kt * 128:
                                           b * T + (kt + 1) * 128],
                                        qT[hl * 64:(hl + 1) * 64,
                                           b * T + qc * 512:
                                           b * T + (qc + 1) * 512],
                                        start=False, stop=True)
                                    ex = expp.tile([128, 512], F32R, tag="ex")
                                    nc.scalar.activation(ex[:], sp[:], AF.Exp)
                                    nc.tensor.matmul(
                                        yus[b * 2 + hl][:],
                                        v_all[:, (b * 2 + hl) * 16 + kt, :],
                                        ex[:],
                                        start=(kt == 0), stop=(kt == KT - 1))
                        for p4 in range(4):
                            b, hl = p4 // 2, p4 % 2
                            rec = nrmp.tile([1, 512], F32, tag="rec")
                            nc.vector.reciprocal(rec[:], yus[p4][64:65, :])
                            drow = p4 * 4 + qc
                            nc.sync.dma_start(
                                denom_dram[drow:drow + 1, :], rec[:])
                            bc = nrmp.tile([64, 512], F32, tag="bc")
                            nc.sync.dma_start(
                                bc[:], bass.AP(tensor=denom_dram,
                                               offset=drow * 512,
                                               ap=[[0, 64], [1, 512]]))
                            nc.vector.tensor_mul(
                                yn[hl][:, b * T + qc * 512:
                                       b * T + (qc + 1) * 512],
                                yus[p4][0:64, :], bc[:])

                with nc.named_scope("a2a"):
                    for j in range(NCORES):
                        nc.sync.dma_start(
                            cc_send[j * 128:j * 128 + 64, :],
                            yn[0][:, j * 512:(j + 1) * 512])
                        nc.sync.dma_start(
                            cc_send[j * 128 + 64:(j + 1) * 128, :],
                            yn[1][:, j * 512:(j + 1) * 512])
                    nc.gpsimd.collective_compute(
                        "AllToAll", mybir.AluOpType.bypass,
                        replica_groups=[list(range(NCORES))],
                        ins=[cc_send[:]], outs=[cc_recv[:]])

            # ---------------- phase 3: out-proj + LN + MLP ----------------
            with nc.named_scope("mlp"), \
                 tc.tile_pool(name="p3w", bufs=1) as p3w, \
                 tc.tile_pool(name="p3acc", bufs=2, space="PSUM") as p3acc, \
                 tc.tile_pool(name="p3mo", bufs=4, space="PSUM") as p3mo, \
                 tc.tile_pool(name="p3pt", bufs=2, space="PSUM") as p3pt, \
                 tc.tile_pool(name="p3sb", bufs=1) as p3sb, \
                 tc.tile_pool(name="p3r", bufs=2) as p3r, \
                 tc.tile_pool(name="p3s", bufs=4) as p3s, \
                 tc.tile_pool(name="mlpw", bufs=3) as mlpw:
                yrecv, wout = [], []
                for kk in range(8):
                    yr = p3w.tile([128, 512], BF, tag=f"yr{kk}")
                    nc.sync.dma_start(yr[:], cc_recv[kk * 128:(kk + 1) * 128, :])
                    yrecv.append(yr)
                for kk in range(8):
                    wo = p3w.tile([128, D], BF, tag=f"wo{kk}")
                    nc.sync.dma_start(wo[:], w_outT_io[kk * 128:(kk + 1) * 128, :])
                    wout.append(wo)
                b_in = p3sb.tile([128, 32], F32, tag="b_in")
                nc.sync.dma_start(b_in[:], b_inT_io[:])

                y_sb = p3sb.tile([128, 4, D], F32, tag="y_sb")
                y2_sb = p3sb.tile([128, 4, D], F32, tag="y2_sb")
                x_res_r = x_res_io.rearrange("(t p) d -> p t d", p=128)
                x_res_b_r = x_res_b_io.rearrange("(t p) d -> p t d", p=128)
                for tt in range(4):
                    xr = p3r.tile([128, D], F32, tag="xr")
                    nc.sync.dma_start(xr[:], x_res_r[:, tt, :])
                    xrb = p3r.tile([128, D], F32, tag="xrb")
                    nc.sync.dma_start(xrb[:], x_res_b_r[:, tt, :])
                    for dc in range(2):
                        ps = p3acc.tile([128, 512], F32, tag="acc")
                        for kk in range(8):
                            nc.tensor.matmul(
                                ps[:], yrecv[kk][:, tt * 128:(tt + 1) * 128],
                                wout[kk][:, dc * 512:(dc + 1) * 512],
                                start=(kk == 0), stop=(kk == 7))
                        nc.vector.tensor_add(
                            y_sb[:, tt, dc * 512:(dc + 1) * 512], ps[:],
                            xr[:, dc * 512:(dc + 1) * 512])
                        nc.vector.tensor_add(
                            y2_sb[:, tt, dc * 512:(dc + 1) * 512], ps[:],
                            xrb[:, dc * 512:(dc + 1) * 512])

                # LayerNorm -> h_norm (bf16) -> transpose -> hT (D-major)
                hT = p3sb.tile([128, 8, 512], BF, tag="hT")
                for tt in range(4):
                    stats = p3s.tile([128, 2, 6], F32, tag="stats")
                    for g in range(2):
                        nc.vector.bn_stats(
                            stats[:, g, :],
                            y_sb[:, tt, g * 512:(g + 1) * 512])
                    mv = p3s.tile([128, 2], F32, tag="mv")
                    nc.vector.bn_aggr(mv[:], stats[:])
                    eps = p3s.tile([128, 1], F32, tag="eps")
                    nc.vector.memset(eps[:], 1e-5)
                    sd = p3s.tile([128, 1], F32, tag="sd")
                    nc.scalar.activation(sd[:], mv[:, 1:2], AF.Sqrt,
                                         bias=eps[:], scale=1.0)
                    rstd = p3s.tile([128, 1], F32, tag="rstd")
                    nc.vector.reciprocal(rstd[:], sd[:])
                    nb = p3s.tile([128, 1], F32, tag="nb")
                    nc.vector.tensor_mul(nb[:], mv[:, 0:1], rstd[:])
                    nb2 = p3s.tile([128, 1], F32, tag="nb2")
                    nc.scalar.mul(nb2[:], nb[:], -1.0)
                    hn = p3r.tile([128, D], BF, tag="hn")
                    nc.scalar.activation(hn[:], y_sb[:, tt, :], AF.Identity,
                                         bias=nb2[:], scale=rstd[:])
                    for dc in range(8):
                        pt = p3pt.tile([128, 128], BF, tag="pt3")
                        nc.tensor.transpose(
                            pt[:], hn[:, dc * 128:(dc + 1) * 128], identb[:])
                        nc.vector.tensor_copy(
                            hT[:, dc, tt * 128:(tt + 1) * 128], pt[:])

                # MLP in + gelu -> hmT (Ff-major bf16)
                hmT = p3sb.tile([128, 32, 512], BF, tag="hmT")
                for ff in range(32):
                    wi = mlpw.tile([128, 8, 128], BF, tag="wi")
                    nc.sync.dma_start(wi[:], w_inP_io[:, ff, :, :])
                    ps = p3acc.tile([128, 512], F32, tag="acc")
                    for kk in range(8):
                        nc.tensor.matmul(ps[:], wi[:, kk, :], hT[:, kk, :],
                                         start=(kk == 0), stop=(kk == 7))
                    nc.scalar.activation(hmT[:, ff, :], ps[:], AF.Gelu,
                                         bias=b_in[:, ff:ff + 1], scale=1.0)

                # MLP out + final residual
                out_r = out_io.rearrange("(t p) d -> p t d", p=128)
                for dc in range(2):
                    pss = [p3mo.tile([128, 512], F32, tag="mo",
                                     name=f"mo{dc}_{i}") for i in range(4)]
                    for ff in range(32):
                        wo2 = mlpw.tile([128, 512], BF, tag="wo2")
                        nc.sync.dma_start(
                            wo2[:], w_mlp_outT_io[ff * 128:(ff + 1) * 128,
                                                  dc * 512:(dc + 1) * 512])
                        for tt in range(4):
                            nc.tensor.matmul(
                                pss[tt][:],
                                hmT[:, ff, tt * 128:(tt + 1) * 128], wo2[:],
                                start=(ff == 0), stop=(ff == 31))
                    for tt in range(4):
                        fin = p3s.tile([128, 512], F32, tag="fin")
                        nc.vector.tensor_add(
                            fin[:], pss[tt][:],
                            y2_sb[:, tt, dc * 512:(dc + 1) * 512])
                        nc.sync.dma_start(
                            out_r[:, tt, dc * 512:(dc + 1) * 512], fin[:])

    nc.compile()
    return nc


def _host_prep(x, alibi, ln1_w, w_qkv, w_out, ln2_w, w_mlp_in, b_mlp_in,
               w_mlp_out, b_mlp_out):
    f32 = np.float32
    x = np.asarray(x, f32)
    x_flat = np.ascontiguousarray(x.reshape(NTOK, D))
    xT = np.ascontiguousarray(x_flat.T)
    w_qkv = np.asarray(w_qkv, f32)
    w_out = np.asarray(w_out, f32)
    w_mlp_in = np.asarray(w_mlp_in, f32)
    w_mlp_out = np.asarray(w_mlp_out, f32)
    b_mlp_in = np.asarray(b_mlp_in, f32)
    b_mlp_out = np.asarray(b_mlp_out, f32)
    ln2_w = np.asarray(ln2_w, f32)
    alibi = np.asarray(alibi, f32)

    w_outT = np.ascontiguousarray(w_out.T).astype(BF16)
    w_in_eff = w_mlp_in * ln2_w[None, :]          # (FF, D)
    # packed [p, ff, kk, fin] = w_in_eff[ff*128+fin, kk*128+p]
    w_inP = np.ascontiguousarray(
        w_in_eff.reshape(32, 128, 8, 128).transpose(3, 0, 2, 1)).astype(BF16)
    w_mlp_outT = np.ascontiguousarray(w_mlp_out.T).astype(BF16)
    b_inT = np.ascontiguousarray(b_mlp_in.reshape(32, 128).T)

    in_maps = []
    for c in range(NCORES):
        h0 = HPC * c
        qrows = w_qkv[h0 * Dh:(h0 + HPC) * Dh] / np.sqrt(np.float32(Dh))
        krows = w_qkv[H * Dh + h0 * Dh:H * Dh + (h0 + HPC) * Dh]
        vrows = w_qkv[2 * H * Dh + h0 * Dh:2 * H * Dh + (h0 + HPC) * Dh]
        wqkvT = np.ascontiguousarray(np.concatenate([qrows, krows, vrows], 0).T)
        alibiT = np.ascontiguousarray(
            np.transpose(alibi[0, h0:h0 + HPC], (0, 2, 1))).astype(BF16)
        x_res = np.ascontiguousarray(x_flat[c * CHUNK:(c + 1) * CHUNK])
        x_res_b = x_res + b_mlp_out[None, :]
        in_maps.append({
            "xT": xT, "wqkvT": wqkvT, "alibiT": alibiT, "w_outT": w_outT,
            "x_res": x_res, "x_res_b": x_res_b, "w_inP": w_inP,
            "b_inT": b_inT, "w_mlp_outT": w_mlp_outT,
        })
    return in_maps


def _get_compiled():
    global _COMPILED
    if _COMPILED is None:
        _COMPILED = _build()
    return _COMPILED


def kernel(_trace=False, **inputs):
    nc = _get_compiled()
    in_maps = _host_prep(**inputs)
    res = run_bass_kernel_spmd(nc, in_maps, core_ids=list(range(NCORES)),
                               trace=_trace)
    out = np.concatenate([res.results[c]["out"] for c in range(NCORES)], 0)
    out = out.reshape(B, T, D).astype(np.float32)
    if _trace:
        return out, res
    return out


# revision 22
# speedup vs baseline: 302.0424x; 302.0424x over previous
"""Trainium2 Bass kernel for nn_AttentionBlock (B=2, T=2048, D=1024, H=16,
Dh=64, Ff=4096), SPMD across 8 NeuronCores in one NEFF launch.

Sharding:
  - Phase 1+2 (QKV projection + attention): 2 heads per core. The alibi
    tensor (256 MiB total) is read bf16, 2 heads per core.
  - AllToAll (1 MiB/core, bf16) re-shards attention output from heads to
    tokens.
  - Phase 3 (out-proj + residual + LayerNorm + MLP): 512 tokens per core.

Numerics:
  - Matmuls on the q/k path use float32r (TF32-like, ~1.5e-4) fed straight
    from fp32 HBM data; bf16 elsewhere (weights, alibi).
  - Attention computes transposed scores S^T(k,q): alibi is injected into
    PSUM via an identity matmul and Q.K^T accumulates on top, so the
    softmax input needs no extra vector work; exp runs on ScalarE
    (PSUM -> SBUF f32r); the softmax denominator falls out of a ones
    column appended to V in the attn@v matmul; 1/denom is broadcast
    across partitions via a DRAM-bounce DMA.
  - Host-side algebraic folds: 1/sqrt(Dh) into w_q, ln2_w into w_mlp_in,
    b_mlp_in via gelu's per-partition bias, b_mlp_out into a second copy
    of the residual.

kernel(**inputs) takes FULL unsharded inputs, returns the FULL output.
"""

import sys

for _p in ("/opt/trn_rl_repo", "/root/.axon_site/_ro/trn_rl_repo"):
    if _p not in sys.path:
        sys.path.insert(0, _p)

import numpy as np
import ml_dtypes

import concourse.bass as bass
import concourse.tile as tile
from concourse import bacc, mybir
from concourse.bass_utils import run_bass_kernel_spmd
from concourse.masks import make_identity

BF16 = ml_dtypes.bfloat16

B, T, D, H, Dh, FF = 2, 2048, 1024, 16, 64, 4096
NTOK = B * T            # 4096
NCORES = 8
CHUNK = NTOK // NCORES  # 512 tokens per core
HPC = H // NCORES       # 2 heads per core

F32 = mybir.dt.float32
F32R = mybir.dt.float32r
BF = mybir.dt.bfloat16
AF = mybir.ActivationFunctionType

_COMPILED = None


def _build(sim1=False):
    nc = bacc.Bacc("TRN2", target_bir_lowering=False, debug=False,
                   num_devices=1 if sim1 else NCORES)

    # ---- kernel I/O (per core) ----
    xT_io = nc.dram_tensor("xT", [D, NTOK], F32R, kind="ExternalInput").ap()
    wqkvT_io = nc.dram_tensor("wqkvT", [D, 384], F32R, kind="ExternalInput").ap()
    alibiT_io = nc.dram_tensor("alibiT", [HPC, T, T], BF, kind="ExternalInput").ap()
    w_outT_io = nc.dram_tensor("w_outT", [D, D], BF, kind="ExternalInput").ap()
    x_res_io = nc.dram_tensor("x_res", [CHUNK, D], F32, kind="ExternalInput").ap()
    x_res_b_io = nc.dram_tensor("x_res_b", [CHUNK, D], F32, kind="ExternalInput").ap()
    # packed as [p, ff, kk, fin] = w_mlp_in_eff[ff*128+fin, kk*128+p]
    w_inP_io = nc.dram_tensor("w_inP", [128, 32, 8, 128], BF, kind="ExternalInput").ap()
    b_inT_io = nc.dram_tensor("b_inT", [128, 32], F32, kind="ExternalInput").ap()
    w_mlp_outT_io = nc.dram_tensor("w_mlp_outT", [FF, D], BF, kind="ExternalInput").ap()
    out_io = nc.dram_tensor("out", [CHUNK, D], F32, kind="ExternalOutput").ap()

    # ---- internal DRAM ----
    cc_send = nc.dram_tensor("cc_send", [D, CHUNK], BF)
    cc_recv = nc.dram_tensor("cc_recv", [D, CHUNK], BF)
    denom_dram = nc.dram_tensor("denom_dram", [8, 1024], F32)

    KT = T // 128   # 16 k-tiles per batch

    with tile.TileContext(nc) as tc:
        with tc.tile_pool(name="consts", bufs=1) as consts:
            identb = consts.tile([128, 128], BF, tag="identb")
            make_identity(nc, identb[:])
            identf = consts.tile([128, 128], F32, tag="identf")
            make_identity(nc, identf[:])
            identr = consts.tile([128, 128], F32R, tag="identr")
            nc.vector.tensor_copy(identr[:], identf[:])


            with tc.tile_pool(name="qkv", bufs=1) as qkv:
                # per-batch q/k/v so batch-1 projection overlaps batch-0
                # attention without false dependencies
                qTs, kTs, vs = [], [], []
                for b in range(2):
                    qTb = qkv.tile([128, T], F32R, tag=f"qT{b}", name=f"qT{b}")
                    kTb = qkv.tile([128, T], F32R, tag=f"kT{b}", name=f"kT{b}")
                    vb = qkv.tile([128, 16, 2, 65], BF, tag=f"v{b}",
                                  name=f"v{b}")
                    nc.vector.memset(vb[:, :, :, 64:65], 1.0)
                    qTs.append(qTb); kTs.append(kTb); vs.append(vb)
                # yn[hl][b*2+qc] covers tokens [b*T + qc*1024, ...)
                yn = [[qkv.tile([64, 1024], BF, tag=f"yn{hl}_{i}",
                                name=f"yn{hl}_{i}") for i in range(4)]
                      for hl in range(2)]

                with tc.tile_pool(name="p1x", bufs=1) as p1x, \
                     tc.tile_pool(name="p1w", bufs=1) as p1w, \
                     tc.tile_pool(name="p1ps", bufs=4, space="PSUM") as p1ps, \
                     tc.tile_pool(name="p1t", bufs=3) as p1t, \
                     tc.tile_pool(name="p1pt", bufs=2, space="PSUM") as p1pt:
                    wq = []
                    for kk in range(8):
                        w = p1w.tile([128, 384], F32R, tag=f"wq{kk}")
                        nc.sync.dma_start(w[:], wqkvT_io[kk * 128:(kk + 1) * 128, :])
                        wq.append(w)
                    for b in range(2):
                        qT, kT, v_all = qTs[b], kTs[b], vs[b]
                        with nc.named_scope(f"qkvproj{b}"):
                            xts = [p1x.tile([128, 2048], F32R,
                                            tag=f"xt{kk}", name=f"xt{kk}_{b}")
                                   for kk in range(8)]
                            for cc4 in range(4):
                                for kk in range(8):
                                    nc.sync.dma_start(
                                        xts[kk][:, cc4 * 512:(cc4 + 1) * 512],
                                        xT_io[kk * 128:(kk + 1) * 128,
                                              b * 2048 + cc4 * 512:
                                              b * 2048 + (cc4 + 1) * 512])
                            for t in range(4):
                                for m in range(3):   # q, k, v
                                    ps = p1ps.tile([128, 512], F32, tag="proj")
                                    for kk in range(8):
                                        nc.tensor.matmul(
                                            ps[:],
                                            wq[kk][:, m * 128:(m + 1) * 128],
                                            xts[kk][:, t * 512:(t + 1) * 512],
                                            start=(kk == 0), stop=(kk == 7))
                                    if m == 0:
                                        nc.vector.tensor_copy(
                                            qT[:, t * 512:(t + 1) * 512], ps[:])
                                    elif m == 1:
                                        nc.vector.tensor_copy(
                                            kT[:, t * 512:(t + 1) * 512], ps[:])
                                    else:
                                        vt = p1t.tile([128, 512], F32R, tag="vt")
                                        nc.vector.tensor_copy(vt[:], ps[:])
                                        for j in range(4):
                                            ti = t * 4 + j
                                            pt = p1pt.tile([128, 128], F32R,
                                                           tag="pt")
                                            nc.tensor.transpose(
                                                pt[:],
                                                vt[:, j * 128:(j + 1) * 128],
                                                identr[:])
                                            nc.vector.tensor_copy(
                                                v_all[:, ti, :, 0:64],
                                                pt[:].rearrange(
                                                    "p (a b) -> p a b", a=2))
                with nc.named_scope("attn"), \
                     tc.tile_pool(name="alb", bufs=4) as albp, \
                     tc.tile_pool(name="exps", bufs=6) as expp, \
                     tc.tile_pool(name="sps", bufs=2, space="PSUM") as spsp, \
                     tc.tile_pool(name="yups", bufs=1, space="PSUM") as yupp, \
                     tc.tile_pool(name="nrm", bufs=3) as nrmp:
                    if True:
                        for hl in range(2):
                            for qc in range(2):   # 1024-wide q chunks
                                yus = [yupp.tile([65, 1024], F32, tag=f"yu{b}",
                                                 name=f"yu{b}_{hl}_{qc}")
                                       for b in range(2)]
                                for kt in range(KT):
                                    al = albp.tile([128, 1024], BF, tag="al")
                                    nc.sync.dma_start(
                                        al[:],
                                        alibiT_io[hl, kt * 128:(kt + 1) * 128,
                                                  qc * 1024:(qc + 1) * 1024])
                                    for b in range(2):
                                        sp = spsp.tile([128, 1024], F32,
                                                       tag="sp")
                                        for h2 in range(2):
                                            nc.tensor.matmul(
                                                sp[:, h2 * 512:(h2 + 1) * 512],
                                                kTs[b][hl * 64:(hl + 1) * 64,
                                                       kt * 128:(kt + 1) * 128],
                                                qTs[b][hl * 64:(hl + 1) * 64,
                                                       qc * 1024 + h2 * 512:
                                                       qc * 1024 + (h2 + 1) * 512],
                                                start=True, stop=True)
                                        ex0 = expp.tile([128, 1024], BF,
                                                        tag="ex0")
                                        nc.scalar.activation(ex0[:], sp[:],
                                                             AF.Exp)
                                        ex = expp.tile([128, 1024], BF,
                                                       tag="ex")
                                        nc.vector.tensor_mul(ex[:], ex0[:],
                                                             al[:])
                                        for h2 in range(2):
                                            nc.tensor.matmul(
                                                yus[b][:, h2 * 512:(h2 + 1) * 512],
                                                vs[b][:, kt, hl, :],
                                                ex[:, h2 * 512:(h2 + 1) * 512],
                                                start=(kt == 0),
                                                stop=(kt == KT - 1))
                                for b in range(2):
                                    rec = nrmp.tile([1, 1024], F32, tag="rec")
                                    nc.vector.reciprocal(rec[:],
                                                         yus[b][64:65, :])
                                    drow = ((b * 2 + hl) * 2 + qc)
                                    nc.sync.dma_start(
                                        denom_dram[drow:drow + 1, :], rec[:])
                                    bc = nrmp.tile([64, 1024], F32, tag="bc")
                                    nc.sync.dma_start(
                                        bc[:], bass.AP(tensor=denom_dram,
                                                       offset=drow * 1024,
                                                       ap=[[0, 64], [1, 1024]]))
                                    nc.vector.tensor_mul(
                                        yn[hl][b * 2 + qc][:],
                                        yus[b][0:64, :], bc[:])

                with nc.named_scope("a2a"):
                    for j in range(NCORES):
                        b, qc, half = j // 4, (j % 4) // 2, j % 2
                        for hl in range(2):
                            nc.sync.dma_start(
                                cc_send[j * 128 + hl * 64:
                                        j * 128 + (hl + 1) * 64, :],
                                yn[hl][b * 2 + qc][:, half * 512:
                                                   (half + 1) * 512])
                    if sim1:
                        nc.sync.dma_start(cc_recv[:], cc_send[:])
                    else:
                        nc.gpsimd.collective_compute(
                            "AllToAll", mybir.AluOpType.bypass,
                            replica_groups=[list(range(NCORES))],
                            ins=[cc_send[:]], outs=[cc_recv[:]])

            # ---------------- phase 3: out-proj + LN + MLP ----------------
            with nc.named_scope("mlp"), \
                 tc.tile_pool(name="p3w", bufs=1) as p3w, \
                 tc.tile_pool(name="p3acc", bufs=2, space="PSUM") as p3acc, \
                 tc.tile_pool(name="p3mo", bufs=4, space="PSUM") as p3mo, \
                 tc.tile_pool(name="p3pt", bufs=2, space="PSUM") as p3pt, \
                 tc.tile_pool(name="p3sb", bufs=1) as p3sb, \
                 tc.tile_pool(name="p3r", bufs=3) as p3r, \
                 tc.tile_pool(name="p3s", bufs=4) as p3s, \
                 tc.tile_pool(name="mlpw", bufs=8) as mlpw:
                yrecv, wout = [], []
                for kk in range(8):
                    yr = p3w.tile([128, 512], BF, tag=f"yr{kk}")
                    nc.sync.dma_start(yr[:], cc_recv[kk * 128:(kk + 1) * 128, :])
                    yrecv.append(yr)
                for kk in range(8):
                    wo = p3w.tile([128, D], BF, tag=f"wo{kk}")
                    nc.sync.dma_start(wo[:], w_outT_io[kk * 128:(kk + 1) * 128, :])
                    wout.append(wo)
                b_in = p3sb.tile([128, 32], F32, tag="b_in")
                nc.sync.dma_start(b_in[:], b_inT_io[:])

                y_sb = p3sb.tile([128, 4, D], F32, tag="y_sb")
                y2_sb = p3sb.tile([128, 4, D], F32, tag="y2_sb")
                x_res_r = x_res_io.rearrange("(t p) d -> p t d", p=128)
                x_res_b_r = x_res_b_io.rearrange("(t p) d -> p t d", p=128)
                for tt in range(4):
                    xr = p3r.tile([128, D], F32, tag="xr")
                    nc.sync.dma_start(xr[:], x_res_r[:, tt, :])
                    xrb = p3r.tile([128, D], F32, tag="xrb")
                    nc.sync.dma_start(xrb[:], x_res_b_r[:, tt, :])
                    for dc in range(2):
                        ps = p3acc.tile([128, 512], F32, tag="acc")
                        for kk in range(8):
                            nc.tensor.matmul(
                                ps[:], yrecv[kk][:, tt * 128:(tt + 1) * 128],
                                wout[kk][:, dc * 512:(dc + 1) * 512],
                                start=(kk == 0), stop=(kk == 7))
                        nc.vector.tensor_add(
                            y_sb[:, tt, dc * 512:(dc + 1) * 512], ps[:],
                            xr[:, dc * 512:(dc + 1) * 512])
                        nc.vector.tensor_add(
                            y2_sb[:, tt, dc * 512:(dc + 1) * 512], ps[:],
                            xrb[:, dc * 512:(dc + 1) * 512])

                # LayerNorm -> h_norm (bf16) -> transpose -> hT (D-major)
                hT = p3sb.tile([128, 8, 512], BF, tag="hT")
                for tt in range(4):
                    stats = p3s.tile([128, 2, 6], F32, tag="stats")
                    for g in range(2):
                        nc.vector.bn_stats(
                            stats[:, g, :],
                            y_sb[:, tt, g * 512:(g + 1) * 512])
                    mv = p3s.tile([128, 2], F32, tag="mv")
                    nc.vector.bn_aggr(mv[:], stats[:])
                    eps = p3s.tile([128, 1], F32, tag="eps")
                    nc.vector.memset(eps[:], 1e-5)
                    sd = p3s.tile([128, 1], F32, tag="sd")
                    nc.scalar.activation(sd[:], mv[:, 1:2], AF.Sqrt,
                                         bias=eps[:], scale=1.0)
                    rstd = p3s.tile([128, 1], F32, tag="rstd")
                    nc.vector.reciprocal(rstd[:], sd[:])
                    nb = p3s.tile([128, 1], F32, tag="nb")
                    nc.vector.tensor_mul(nb[:], mv[:, 0:1], rstd[:])
                    nb2 = p3s.tile([128, 1], F32, tag="nb2")
                    nc.scalar.mul(nb2[:], nb[:], -1.0)
                    hn = p3r.tile([128, D], BF, tag="hn")
                    nc.scalar.activation(hn[:], y_sb[:, tt, :], AF.Identity,
                                         bias=nb2[:], scale=rstd[:])
                    for dc in range(8):
                        pt = p3pt.tile([128, 128], BF, tag="pt3")
                        nc.tensor.transpose(
                            pt[:], hn[:, dc * 128:(dc + 1) * 128], identb[:])
                        nc.vector.tensor_copy(
                            hT[:, dc, tt * 128:(tt + 1) * 128], pt[:])

                # MLP in + gelu -> hmT (Ff-major bf16)
                hmT = p3sb.tile([128, 32, 512], BF, tag="hmT")
                for ff in range(32):
                    wi = mlpw.tile([128, 8, 128], BF, tag="wi")
                    nc.sync.dma_start(wi[:], w_inP_io[:, ff, :, :])
                    ps = p3acc.tile([128, 512], F32, tag="acc")
                    for kk in range(8):
                        nc.tensor.matmul(ps[:], wi[:, kk, :], hT[:, kk, :],
                                         start=(kk == 0), stop=(kk == 7))
                    nc.scalar.activation(hmT[:, ff, :], ps[:], AF.Gelu,
                                         bias=b_in[:, ff:ff + 1], scale=1.0)

                # MLP out + final residual
                out_r = out_io.rearrange("(t p) d -> p t d", p=128)
                for dc in range(2):
                    pss = [p3mo.tile([128, 512], F32, tag="mo",
                                     name=f"mo{dc}_{i}") for i in range(4)]
                    for ff in range(32):
                        wo2 = mlpw.tile([128, 512], BF, tag="wo2")
                        nc.sync.dma_start(
                            wo2[:], w_mlp_outT_io[ff * 128:(ff + 1) * 128,
                                                  dc * 512:(dc + 1) * 512])
                        for tt in range(4):
                            nc.tensor.matmul(
                                pss[tt][:],
                                hmT[:, ff, tt * 128:(tt + 1) * 128], wo2[:],
                                start=(ff == 0), stop=(ff == 31))
                    for tt in range(4):
                        fin = p3s.tile([128, 512], F32, tag="fin")
                        nc.vector.tensor_add(
                            fin[:], pss[tt][:],
                            y2_sb[:, tt, dc * 512:(dc + 1) * 512])
                        nc.sync.dma_start(
                            out_r[:, tt, dc * 512:(dc + 1) * 512], fin[:])

    nc.compile()
    return nc


def _host_prep(x, alibi, ln1_w, w_qkv, w_out, ln2_w, w_mlp_in, b_mlp_in,
               w_mlp_out, b_mlp_out):
    f32 = np.float32
    x = np.asarray(x, f32)
    x_flat = np.ascontiguousarray(x.reshape(NTOK, D))
    xT = np.ascontiguousarray(x_flat.T)
    w_qkv = np.asarray(w_qkv, f32)
    w_out = np.asarray(w_out, f32)
    w_mlp_in = np.asarray(w_mlp_in, f32)
    w_mlp_out = np.asarray(w_mlp_out, f32)
    b_mlp_in = np.asarray(b_mlp_in, f32)
    b_mlp_out = np.asarray(b_mlp_out, f32)
    ln2_w = np.asarray(ln2_w, f32)
    alibi = np.asarray(alibi, f32)

    w_outT = np.ascontiguousarray(w_out.T).astype(BF16)
    w_in_eff = w_mlp_in * ln2_w[None, :]          # (FF, D)
    # packed [p, ff, kk, fin] = w_in_eff[ff*128+fin, kk*128+p]
    w_inP = np.ascontiguousarray(
        w_in_eff.reshape(32, 128, 8, 128).transpose(3, 0, 2, 1)).astype(BF16)
    w_mlp_outT = np.ascontiguousarray(w_mlp_out.T).astype(BF16)
    b_inT = np.ascontiguousarray(b_mlp_in.reshape(32, 128).T)

    in_maps = []
    for c in range(NCORES):
        h0 = HPC * c
        qrows = w_qkv[h0 * Dh:(h0 + HPC) * Dh] / np.sqrt(np.float32(Dh))
        krows = w_qkv[H * Dh + h0 * Dh:H * Dh + (h0 + HPC) * Dh]
        vrows = w_qkv[2 * H * Dh + h0 * Dh:2 * H * Dh + (h0 + HPC) * Dh]
        wqkvT = np.ascontiguousarray(np.concatenate([qrows, krows, vrows], 0).T)
        alibiT = np.exp(np.ascontiguousarray(
            np.transpose(alibi[0, h0:h0 + HPC], (0, 2, 1)))).astype(BF16)
        x_res = np.ascontiguousarray(x_flat[c * CHUNK:(c + 1) * CHUNK])
        x_res_b = x_res + b_mlp_out[None, :]
        in_maps.append({
            "xT": xT, "wqkvT": wqkvT, "alibiT": alibiT, "w_outT": w_outT,
            "x_res": x_res, "x_res_b": x_res_b, "w_inP": w_inP,
            "b_inT": b_inT, "w_mlp_outT": w_mlp_outT,
        })
    return in_maps


def _get_compiled():
    global _COMPILED
    if _COMPILED is None:
        _COMPILED = _build()
    return _COMPILED


def kernel(_trace=False, **inputs):
    nc = _get_compiled()
    in_maps = _host_prep(**inputs)
    res = run_bass_kernel_spmd(nc, in_maps, core_ids=list(range(NCORES)),
                               trace=_trace)
    out = np.concatenate([res.results[c]["out"] for c in range(NCORES)], 0)
    out = out.reshape(B, T, D).astype(np.float32)
    if _trace:
        return out, res
    return out
